# revision 18
# baseline (speedup 1.0000x reference)
"""Trainium2 Bass kernel for nn_Encoder_31550829756513 (2-layer dual-branch GCN).

Strategy (8 NeuronCores, node-partitioned graph parallel):
  - Host: build sym-norms for both branches, append self-loop pseudo-edges,
    sort edges by (destination block, source), pack per-(core, slot, range)
    128-edge columns with a shared compile-time column schedule. Source
    indices are split into 32768-row ranges so they fit dma_gather's int16
    index format.
  - Device, per core (single SPMD program, all offsets static):
      phase B: full hw = x @ W1 table computed locally on every core
          (x is a full input; avoids an AllGather entirely)
      L1: per (group, range): one batched dma_gather (row i -> partition
          i%128, column i//128); per column: two fused selector ops
          (iota==dst)*norm split across DVE and GpSimd, two PE matmuls
          accumulate z_g|z_p in PSUM; per block: h1 = relu(z + b1), then
          premultiply by W2 on-device (PE transpose + matmul) so only the
          O-wide table t = h1 @ W2 [NPAD, 2*O] fp16 is AllGathered.
      AllGather t -> full fp16 table [NPAD, 2*O]
      L2: same message pass on t; per block: logits via dot with dense_w,
          softmax-of-2 == sigmoid(lg-lp), blend in O-space, + b2 -> out.
  - Host: concatenate output shards, slice to N rows.
"""

import os
import numpy as np

P = 128
R15 = 32768          # dma_gather int16 index range
NR = 4               # number of index ranges covering NPAD
PREP_VER = 6

_FP16 = np.float16


class Cfg:
    def __init__(self, n, e, d=256, h=128, o=64, ncores=8, gb=7, gg=5):
        self.N = n
        self.E = e
        self.D = d
        self.H = h
        self.O = o
        self.ncores = ncores
        self.NBLK = -(-n // P)
        self.NB = -(-self.NBLK // ncores)
        self.CORE_ROWS = self.NB * P
        self.NPAD = ncores * self.CORE_ROWS
        self.NBLK_ALL = self.NPAD // P
        self.GB = gb
        self.GG = gg


FULL = Cfg(100000, 1600000)


def _schedule(cfg, sched):
    """Expand the shared schedule tuple.

    sched = (Ktup, passes_tup):
      Ktup: per slot, number of columns
      passes_tup: per group: tuple of (col_start, ncols, base) dma_gather
        passes (window [base, base+R15) in table rows)
    Returns (groups, passes, block_cols, NCH).
    """
    K, passes_tup = sched
    NB, GG = cfg.NB, cfg.GG
    groups = [(g0, min(GG, NB - g0)) for g0 in range(0, NB, GG)]
    block_cols = [[] for _ in range(NB)]
    col = 0
    maxKs = []
    for (g0, gs) in groups:
        mk = max(K[s] for s in range(g0, g0 + gs))
        maxKs.append(mk)
        for k in range(mk):
            for s in range(g0, g0 + gs):
                if k < K[s]:
                    block_cols[s].append(col)
                    col += 1
    passes = [list(p) for p in passes_tup]
    return groups, passes, block_cols, col


# ----------------------------------------------------------------------------
# Host preprocessing
# ----------------------------------------------------------------------------

def _preprocess(cfg, x, edge_index, ppmi_edge_weight, W1, b1, W2, b2,
                dense_w, dense_b):
    n, e = cfg.N, cfg.E
    row = np.asarray(edge_index[0], dtype=np.int64).astype(np.int32)
    col = np.asarray(edge_index[1], dtype=np.int64).astype(np.int32)
    ppmi = np.asarray(ppmi_edge_weight, dtype=np.float64)

    sl = np.arange(n, dtype=np.int32)
    row_sl = np.concatenate([row, sl])
    ones_n = np.ones(n, dtype=np.float64)

    def sym_dis(ew):
        deg = np.bincount(row_sl, weights=ew, minlength=n)
        return np.where(deg > 0, deg ** -0.5, 0.0)

    dis_g = sym_dis(np.concatenate([np.ones(e), ones_n]))
    dis_p = sym_dis(np.concatenate([ppmi, ones_n]))

    # augmented edge list: real edges + self-loop pseudo-edges
    src_a = np.concatenate([row, sl])
    dst_a = np.concatenate([col, sl])
    gn_a = np.concatenate([dis_g[row] * dis_g[col],
                           (dis_g * dis_g)]).astype(np.float32)
    pn_a = np.concatenate([dis_p[row] * ppmi * dis_p[col],
                           (dis_p * dis_p)]).astype(np.float32)

    blk_all = dst_a >> 7
    order = np.lexsort((src_a, blk_all))  # dst block, then ascending src
    src_s = src_a[order].astype(np.int64)
    dst_s = dst_a[order]
    gn_s = gn_a[order]
    pn_s = pn_a[order]
    blk = blk_all[order]
    dstloc = (dst_s & 127).astype(np.float32)

    core_of = blk // cfg.NB
    slot_of = blk - core_of * cfg.NB

    bcnt = np.bincount(blk, minlength=cfg.NBLK_ALL)
    run_start_of_blk = np.concatenate([[0], np.cumsum(bcnt)[:-1]])
    pos = np.arange(src_s.shape[0], dtype=np.int64)

    # Shared per-slot source bands: band k of slot s holds <=128 edges for
    # every core, so the k-th column of all cores covers one narrow source
    # window (required for shared dma_gather pass bases).
    INF = np.iinfo(np.int64).max
    K = np.zeros(cfg.NB, dtype=np.int64)
    bounds_of_slot = []
    for s_ in range(cfg.NB):
        arrs = []
        for c_ in range(cfg.ncores):
            b_ = c_ * cfg.NB + s_
            arrs.append(src_s[run_start_of_blk[b_]:
                              run_start_of_blk[b_] + bcnt[b_]])
        ptrs = [0] * cfg.ncores
        bounds = [0]
        while any(ptrs[c_] < arrs[c_].shape[0]
                  for c_ in range(cfg.ncores)):
            v = INF
            for c_ in range(cfg.ncores):
                if ptrs[c_] + P < arrs[c_].shape[0]:
                    v = min(v, int(arrs[c_][ptrs[c_] + P]))
            if v == INF:
                bounds.append(cfg.NPAD)
                break
            assert v > bounds[-1], "band stall: >128 same-src edges"
            for c_ in range(cfg.ncores):
                ptrs[c_] = int(np.searchsorted(arrs[c_], v, side="left"))
            bounds.append(v)
        if len(bounds) == 1:
            bounds.append(cfg.NPAD)
        bounds_of_slot.append(np.asarray(bounds, dtype=np.int64))
        K[s_] = len(bounds) - 1

    # per-edge band index and rank within (core, block, band)
    kcol = np.empty(src_s.shape[0], dtype=np.int64)
    rank_in_band = np.empty(src_s.shape[0], dtype=np.int64)
    for b_ in range(cfg.NBLK_ALL):
        lo_, n_ = run_start_of_blk[b_], bcnt[b_]
        if n_ == 0:
            continue
        s_ = b_ % cfg.NB
        seg = src_s[lo_:lo_ + n_]
        kc = np.searchsorted(bounds_of_slot[s_], seg, side="right") - 1
        band_start = np.searchsorted(seg, bounds_of_slot[s_][kc],
                                     side="left")
        kcol[lo_:lo_ + n_] = kc
        rank_in_band[lo_:lo_ + n_] = np.arange(n_) - band_start
    lane = rank_in_band
    assert (lane >= 0).all() and (lane < P).all()

    # per (slot, k) global src min/max across cores
    Kmax = int(K.max())
    cmin = np.full((cfg.NB, Kmax), INF, dtype=np.int64)
    cmax = np.full((cfg.NB, Kmax), -1, dtype=np.int64)
    np.minimum.at(cmin, (slot_of, kcol), src_s)
    np.maximum.at(cmax, (slot_of, kcol), src_s)

    # schedule: group columns ordered (k, slot); greedy window passes
    GUARD = 64
    groups = [(g0, min(cfg.GG, cfg.NB - g0))
              for g0 in range(0, cfg.NB, cfg.GG)]
    passes_l = []
    colpos = np.full((cfg.NB, Kmax), -1, dtype=np.int64)
    col = 0
    for (g0, gs) in groups:
        mk = max(int(K[s]) for s in range(g0, g0 + gs))
        gp = []
        cur = None  # [start, ncols, lo, hi]
        for k in range(mk):
            for sl in range(g0, g0 + gs):
                if k >= K[sl]:
                    continue
                colpos[sl][k] = col
                lo, hi = int(cmin[sl][k]), int(cmax[sl][k])
                pad = hi < 0
                if cur is None:
                    cur = [col, 1, lo, hi] if not pad else [col, 1, None,
                                                           None]
                elif pad:
                    cur[1] += 1
                else:
                    nlo = lo if cur[2] is None else min(cur[2], lo)
                    nhi = hi if cur[3] is None else max(cur[3], hi)
                    if nhi - nlo < R15 - GUARD:
                        cur[1] += 1
                        cur[2], cur[3] = nlo, nhi
                    else:
                        gp.append((cur[0], cur[1],
                                   int(cur[2] or 0)))
                        cur = [col, 1, lo, hi]
                col += 1
        assert cur is not None
        gp.append((cur[0], cur[1], int(cur[2] or 0)))
        for (st, nc_, ba) in gp:
            pass  # sanity below
        passes_l.append(tuple(gp))
    NCH = col
    for (g0, gs), gp in zip(groups, passes_l):
        for (st, ncols, base) in gp:
            assert base + R15 <= cfg.NPAD or True
    sched = (tuple(int(v) for v in K), tuple(passes_l))

    # place edges
    abscol = colpos[slot_of, kcol]
    c_arr = core_of.astype(np.int64)

    dst_stream = np.zeros((cfg.ncores, P, NCH), dtype=np.float32)
    nrm_stream = np.zeros((cfg.ncores, P, NCH, 2), dtype=np.float32)
    dst_stream[c_arr, lane, abscol] = dstloc
    nrm_stream[c_arr, lane, abscol, 0] = gn_s
    nrm_stream[c_arr, lane, abscol, 1] = pn_s

    # int16 index stream in dma_gather wrap layout
    pass_start_of_col = np.zeros(NCH, dtype=np.int64)
    pass_base_of_col = np.zeros(NCH, dtype=np.int64)
    for gp in passes_l:
        for (st, ncols, base) in gp:
            pass_start_of_col[st:st + ncols] = st
            pass_base_of_col[st:st + ncols] = base
    idx16 = np.zeros((cfg.ncores, 16, NCH * 8), dtype=np.int16)
    ps_e = pass_start_of_col[abscol]
    rel = src_s - pass_base_of_col[abscol]
    assert (rel >= 0).all() and (rel < R15).all()
    i_flat = (abscol - ps_e) * P + lane
    idx16[c_arr, i_flat % 16, ps_e * 8 + i_flat // 16] = rel.astype(np.int16)
    idx16_full = np.ascontiguousarray(np.tile(idx16, (1, 8, 1)))

    xT = np.zeros((cfg.D, cfg.NPAD), dtype=_FP16)
    xT[:, :n] = np.asarray(x, dtype=np.float32).T.astype(_FP16)

    W1f = np.asarray(W1, dtype=np.float32).astype(_FP16)
    W2f = np.asarray(W2, dtype=np.float32).astype(_FP16)
    b1r2 = np.tile(np.asarray(b1, dtype=np.float32)[None, :], (P, 2))
    dwb = np.tile(np.asarray(dense_w, dtype=np.float32).ravel()[None, :],
                  (P, 1))
    b2r = np.tile(np.asarray(b2, dtype=np.float32)[None, :], (P, 1))

    in_maps = []
    for c in range(cfg.ncores):
        in_maps.append({
            "xT": xT,
            "w1": W1f, "w2": W2f, "b1r2": b1r2, "dwb": dwb, "b2r": b2r,
            "idx16": idx16_full[c], "dsts": dst_stream[c],
            "nrms": nrm_stream[c],
        })
    return in_maps, sched


# ----------------------------------------------------------------------------
# Device program
# ----------------------------------------------------------------------------

def build_program(cfg, sched):
    from concourse import bass, mybir, tile, bacc
    from concourse.masks import make_identity

    dt16 = mybir.dt.float16
    dt32 = mybir.dt.float32
    AOT = mybir.AluOpType
    AFT = mybir.ActivationFunctionType

    groups, passes, block_cols, NCH = _schedule(cfg, sched)
    NB, H, O, D = cfg.NB, cfg.H, cfg.O, cfg.D
    O2 = 2 * O

    nc = bacc.Bacc("TRN2", debug=False, enable_asserts=False,
                   num_devices=cfg.ncores)

    xT = nc.dram_tensor("xT", [D, cfg.NPAD], dt16, kind="ExternalInput")
    w1 = nc.dram_tensor("w1", [D, H], dt16, kind="ExternalInput")
    w2 = nc.dram_tensor("w2", [H, O], dt16, kind="ExternalInput")
    b1r2 = nc.dram_tensor("b1r2", [P, 2 * H], dt32, kind="ExternalInput")
    dwb = nc.dram_tensor("dwb", [P, O], dt32, kind="ExternalInput")
    b2r = nc.dram_tensor("b2r", [P, O], dt32, kind="ExternalInput")
    idx16 = nc.dram_tensor("idx16", [P, NCH * 8], mybir.dt.int16,
                           kind="ExternalInput")
    dsts = nc.dram_tensor("dsts", [P, NCH], dt32, kind="ExternalInput")
    nrms = nc.dram_tensor("nrms", [P, NCH, 2], dt32, kind="ExternalInput")
    outp = nc.dram_tensor("out", [cfg.CORE_ROWS, O], dt32,
                          kind="ExternalOutput")

    hw_full = nc.dram_tensor("hw_full", [cfg.NPAD, H], dt16)
    t_shard = nc.dram_tensor("t_shard", [cfg.CORE_ROWS, O2], dt16)
    t_full = nc.dram_tensor("t_full", [cfg.NPAD, O2], dt16,
                            addr_space="Shared")

    groups_all = [list(range(cfg.ncores))]
    seln = [0]

    with tile.TileContext(nc) as tc:
        with tc.tile_pool(name="const", bufs=1) as cpool:
            w1a = cpool.tile([P, H], dt16)
            w1b = cpool.tile([P, H], dt16)
            nc.sync.dma_start(out=w1a[:], in_=w1[0:P, :])
            nc.sync.dma_start(out=w1b[:], in_=w1[P:2 * P, :])
            w2sb = cpool.tile([P, O], dt16)
            nc.sync.dma_start(out=w2sb[:], in_=w2[:, :])
            b1sb = cpool.tile([P, 2 * H], dt32)
            nc.sync.dma_start(out=b1sb[:], in_=b1r2[:, :])
            dwsb = cpool.tile([P, O], dt32)
            nc.sync.dma_start(out=dwsb[:], in_=dwb[:, :])
            b2sb = cpool.tile([P, O], dt32)
            nc.sync.dma_start(out=b2sb[:], in_=b2r[:, :])
            it16 = cpool.tile([P, P], mybir.dt.int16)
            nc.gpsimd.iota(it16[:], pattern=[[1, P]], base=0,
                           channel_multiplier=0)
            iotaf = cpool.tile([P, P], dt16)
            nc.vector.tensor_copy(out=iotaf[:], in_=it16[:])
            ident16 = cpool.tile([P, P], dt16)
            make_identity(nc, ident16[:])
            idx_sb = cpool.tile([P, NCH * 8], mybir.dt.int16)
            nc.sync.dma_start(out=idx_sb[:], in_=idx16[:, :])
            dst_sb = cpool.tile([P, NCH], dt32)
            nc.sync.dma_start(out=dst_sb[:], in_=dsts[:, :])
            nrm_sb = cpool.tile([P, NCH, 2], dt32)
            nc.sync.dma_start(out=nrm_sb[:], in_=nrms[:, :, :])

            def build_sel(spool, c, br):
                sb = spool.tile([P, P], dt16, tag="sel")
                eng = nc.gpsimd if (seln[0] % 4 == 3) else nc.vector
                seln[0] += 1
                eng.tensor_scalar(
                    out=sb[:], in0=iotaf[:],
                    scalar1=dst_sb[:, c:c + 1],
                    scalar2=nrm_sb[:, c, br:br + 1],
                    op0=AOT.is_equal, op1=AOT.mult)
                return sb

            def gather_group(gpool, gi, table, tbl_rows, tag):
                gp = passes[gi]
                base = gp[0][0]
                kg = sum(ncols for (_, ncols, _) in gp)
                gth = gpool.tile([P, kg, P], dt16, tag=tag)
                for (start, ncols, wbase) in gp:
                    hi = min(wbase + R15, tbl_rows)
                    nc.gpsimd.dma_gather(
                        out_ap=gth[:, start - base:start - base + ncols, :],
                        in_ap=table.ap()[wbase:hi, :],
                        idxs_ap=idx_sb[:, start * 8:(start + ncols) * 8],
                        num_idxs=ncols * P,
                        num_idxs_reg=ncols * P,
                        elem_size=P,
                    )
                return gth, base

            # ------------- phase B: full hw = x @ W1 on every core ----------
            hw_view = hw_full.ap().rearrange("(t p) h -> p t h", p=P)
            with tc.tile_pool(name="phB", bufs=3) as xp, \
                 tc.tile_pool(name="phBp", bufs=4, space="PSUM") as bp, \
                 tc.tile_pool(name="phBh", bufs=2) as hp:
                for g0 in range(0, cfg.NBLK_ALL, cfg.GB):
                    gs = min(cfg.GB, cfg.NBLK_ALL - g0)
                    xa = xp.tile([P, gs * P], dt16, tag="xa")
                    xb = xp.tile([P, gs * P], dt16, tag="xb")
                    nc.sync.dma_start(out=xa[:],
                                      in_=xT[0:P, g0 * P:(g0 + gs) * P])
                    nc.sync.dma_start(out=xb[:],
                                      in_=xT[P:2 * P, g0 * P:(g0 + gs) * P])
                    hwg = hp.tile([P, gs, H], dt16, tag="hwg")
                    for t in range(gs):
                        ps = bp.tile([P, H], dt32, tag="bps")
                        nc.tensor.matmul(out=ps[:],
                                         lhsT=xa[:, t * P:(t + 1) * P],
                                         rhs=w1a[:], start=True, stop=False)
                        nc.tensor.matmul(out=ps[:],
                                         lhsT=xb[:, t * P:(t + 1) * P],
                                         rhs=w1b[:], start=False, stop=True)
                        if t % 2 == 0:
                            nc.scalar.activation(out=hwg[:, t, :], in_=ps[:],
                                                 func=AFT.Copy)
                        else:
                            nc.vector.tensor_copy(out=hwg[:, t, :], in_=ps[:])
                    nc.sync.dma_start(out=hw_view[:, g0:g0 + gs, :], in_=hwg[:])

            # ------------- L1 message pass + on-device @W2 ------------------
            t_view = t_shard.ap().rearrange("(t p) o -> p t o", p=P)
            with tc.tile_pool(name="d_g", bufs=3) as gpool, \
                 tc.tile_pool(name="d_z", bufs=3, space="PSUM") as zpool, \
                 tc.tile_pool(name="d_s", bufs=48) as spool, \
                 tc.tile_pool(name="d_h", bufs=3) as hpool, \
                 tc.tile_pool(name="d_tp", bufs=2, space="PSUM") as tppool, \
                 tc.tile_pool(name="d_tm", bufs=2, space="PSUM") as tmpool, \
                 tc.tile_pool(name="d_o", bufs=2) as opool:
                for gi, (g0, gs) in enumerate(groups):
                    gth, base = gather_group(gpool, gi, hw_full, cfg.NPAD,
                                             "gth1")
                    og = opool.tile([P, gs, O2], dt16, tag="og1")
                    for s in range(gs):
                        blk = g0 + s
                        cols = block_cols[blk]
                        z = zpool.tile([P, 2 * H], dt32, tag="z")
                        for br in range(2):
                            for j, c in enumerate(cols):
                                sb = build_sel(spool, c, br)
                                nc.tensor.matmul(
                                    out=z[:, br * H:(br + 1) * H],
                                    lhsT=sb[:], rhs=gth[:, c - base, :],
                                    start=(j == 0),
                                    stop=(j == len(cols) - 1))
                        hb = hpool.tile([P, 2, H], dt16, tag="hb")
                        nc.vector.tensor_tensor(
                            out=hb[:].rearrange("p b h -> p (b h)"),
                            in0=z[:], in1=b1sb[:], op=AOT.add)
                        hr = hpool.tile([P, 2, H], dt16, tag="hr")
                        nc.scalar.activation(
                            out=hr[:].rearrange("p b h -> p (b h)"),
                            in_=hb[:].rearrange("p b h -> p (b h)"),
                            func=AFT.Relu)
                        tp = tppool.tile([P, 2, P], dt16, tag="tp")
                        nc.tensor.transpose(out=tp[:, 0, :], in_=hr[:, 0, :],
                                            identity=ident16[:])
                        nc.tensor.transpose(out=tp[:, 1, :], in_=hr[:, 1, :],
                                            identity=ident16[:])
                        hT = hpool.tile([P, 2, P], dt16, tag="hT")
                        nc.scalar.activation(
                            out=hT[:].rearrange("p b q -> p (b q)"),
                            in_=tp[:].rearrange("p b q -> p (b q)"),
                            func=AFT.Copy)
                        tm = tmpool.tile([P, 2, O], dt32, tag="tm")
                        nc.tensor.matmul(out=tm[:, 0, :], lhsT=hT[:, 0, :],
                                         rhs=w2sb[:], start=True, stop=True)
                        nc.tensor.matmul(out=tm[:, 1, :], lhsT=hT[:, 1, :],
                                         rhs=w2sb[:], start=True, stop=True)
                        nc.scalar.activation(
                            out=og[:, s, :],
                            in_=tm[:].rearrange("p b o -> p (b o)"),
                            func=AFT.Copy)
                    nc.sync.dma_start(
                        out=t_view[:, g0:g0 + gs, :], in_=og[:])

            nc.gpsimd.collective_compute(
                "AllGather", AOT.bypass, replica_groups=groups_all,
                ins=[t_shard.ap().opt()], outs=[t_full.ap().opt()])

            # ------------- L2 message pass + combine ------------------------
            out_view = outp.ap().rearrange("(t p) o -> p t o", p=P)
            with tc.tile_pool(name="f_g", bufs=3) as gpool, \
                 tc.tile_pool(name="f_z", bufs=3, space="PSUM") as zpool, \
                 tc.tile_pool(name="f_s", bufs=48) as spool, \
                 tc.tile_pool(name="f_e", bufs=4) as epool, \
                 tc.tile_pool(name="f_o", bufs=2) as opool:
                for gi, (g0, gs) in enumerate(groups):
                    gth, base = gather_group(gpool, gi, t_full, cfg.NPAD,
                                             "gth2")
                    og = opool.tile([P, gs, O], dt32, tag="og2")
                    for s in range(gs):
                        blk = g0 + s
                        cols = block_cols[blk]
                        zo = zpool.tile([P, O2], dt32, tag="zo")
                        for br in range(2):
                            for j, c in enumerate(cols):
                                sb = build_sel(spool, c, br)
                                nc.tensor.matmul(
                                    out=zo[:, br * O:(br + 1) * O],
                                    lhsT=sb[:],
                                    rhs=gth[:, c - base,
                                            br * O:(br + 1) * O],
                                    start=(j == 0),
                                    stop=(j == len(cols) - 1))
                        zps = epool.tile([P, O], dt32, tag="zps")
                        nc.scalar.activation(out=zps[:], in_=zo[:, O:O2],
                                             func=AFT.Copy)
                        zd = epool.tile([P, O], dt32, tag="zd")
                        nc.vector.tensor_tensor(out=zd[:], in0=zo[:, 0:O],
                                                in1=zps[:],
                                                op=AOT.subtract)
                        scr = epool.tile([P, O], dt32, tag="scr")
                        dl = epool.tile([P, 1], dt32, tag="dl")
                        nc.vector.tensor_tensor(out=scr[:], in0=zd[:],
                                                in1=dwsb[:], op=AOT.mult)
                        nc.vector.tensor_reduce(out=dl[:], in_=scr[:],
                                                axis=mybir.AxisListType.X,
                                                op=AOT.add)
                        wg = epool.tile([P, 1], dt32, tag="wg")
                        nc.scalar.activation(out=wg[:], in_=dl[:],
                                             func=AFT.Sigmoid)
                        bl = epool.tile([P, O], dt32, tag="bl")
                        nc.vector.scalar_tensor_tensor(
                            out=bl[:], in0=zd[:], scalar=wg[:],
                            in1=zps[:], op0=AOT.mult, op1=AOT.add)
                        nc.vector.tensor_tensor(out=og[:, s, :], in0=bl[:],
                                                in1=b2sb[:], op=AOT.add)
                    nc.sync.dma_start(out=out_view[:, g0:g0 + gs, :],
                                      in_=og[:])

    nc.compile()
    return nc


_BUILD_CACHE = {}


def _get_program(cfg, sched):
    key = (cfg.N, cfg.E, cfg.GG, sched)
    if key not in _BUILD_CACHE:
        _BUILD_CACHE[key] = build_program(cfg, sched)
    return _BUILD_CACHE[key]


LAST_RESULTS = None


def _run(cfg, inputs):
    from concourse.bass_utils import run_bass_kernel_spmd
    global LAST_RESULTS
    in_maps, sched = _preprocess(cfg, **inputs)
    nc = _get_program(cfg, sched)
    trace = bool(int(os.environ.get("KERNEL_TRACE", "0")))
    res = run_bass_kernel_spmd(nc, in_maps, core_ids=list(range(cfg.ncores)),
                               trace=trace)
    LAST_RESULTS = res
    out = np.concatenate([res.results[c]["out"] for c in range(cfg.ncores)],
                         axis=0)[:cfg.N]
    return np.ascontiguousarray(out.astype(np.float32))


def kernel(x, edge_index, ppmi_edge_weight, W1, b1, W2, b2, dense_w, dense_b):
    return _run(FULL, dict(x=x, edge_index=edge_index,
                           ppmi_edge_weight=ppmi_edge_weight, W1=W1, b1=b1,
                           W2=W2, b2=b2, dense_w=dense_w, dense_b=dense_b))


# revision 19
# speedup vs baseline: 1.2433x; 1.2433x over previous
"""Trainium2 Bass kernel for nn_Encoder_31550829756513 (2-layer dual-branch GCN).

Strategy (8 NeuronCores, node-partitioned graph parallel):
  - Host: build sym-norms for both branches, append self-loop pseudo-edges,
    sort edges by (destination block, source), pack per-(core, slot, range)
    128-edge columns with a shared compile-time column schedule. Source
    indices are split into 32768-row ranges so they fit dma_gather's int16
    index format.
  - Device, per core (single SPMD program, all offsets static):
      phase B: full hw = x @ W1 table computed locally on every core
          (x is a full input; avoids an AllGather entirely)
      L1: per (group, range): one batched dma_gather (row i -> partition
          i%128, column i//128); per column: two fused selector ops
          (iota==dst)*norm split across DVE and GpSimd, two PE matmuls
          accumulate z_g|z_p in PSUM; per block: h1 = relu(z + b1), then
          premultiply by W2 on-device (PE transpose + matmul) so only the
          O-wide table t = h1 @ W2 [NPAD, 2*O] fp16 is AllGathered.
      AllGather t -> full fp16 table [NPAD, 2*O]
      L2: same message pass on t; per block: logits via dot with dense_w,
          softmax-of-2 == sigmoid(lg-lp), blend in O-space, + b2 -> out.
  - Host: concatenate output shards, slice to N rows.
"""

import os
import numpy as np

P = 128
R15 = 32768          # dma_gather int16 index range
NR = 4               # number of index ranges covering NPAD
PREP_VER = 6

_FP16 = np.float16


class Cfg:
    def __init__(self, n, e, d=256, h=128, o=64, ncores=8, gb=7, gg=5):
        self.N = n
        self.E = e
        self.D = d
        self.H = h
        self.O = o
        self.ncores = ncores
        self.NBLK = -(-n // P)
        self.NB = -(-self.NBLK // ncores)
        self.CORE_ROWS = self.NB * P
        self.NPAD = ncores * self.CORE_ROWS
        self.NBLK_ALL = self.NPAD // P
        self.GB = gb
        self.GG = gg


FULL = Cfg(100000, 1600000)


def _schedule(cfg, sched):
    """Expand the shared schedule tuple.

    sched = (Ktup, passes_tup):
      Ktup: per slot, number of columns
      passes_tup: per group: tuple of (col_start, ncols, base) dma_gather
        passes (window [base, base+R15) in table rows)
    Returns (groups, passes, block_cols, NCH).
    """
    K, passes_tup = sched
    NB, GG = cfg.NB, cfg.GG
    groups = [(g0, min(GG, NB - g0)) for g0 in range(0, NB, GG)]
    block_cols = [[] for _ in range(NB)]
    col = 0
    maxKs = []
    for (g0, gs) in groups:
        mk = max(K[s] for s in range(g0, g0 + gs))
        maxKs.append(mk)
        for k in range(mk):
            for s in range(g0, g0 + gs):
                if k < K[s]:
                    block_cols[s].append(col)
                    col += 1
    passes = [list(p) for p in passes_tup]
    return groups, passes, block_cols, col


# ----------------------------------------------------------------------------
# Host preprocessing
# ----------------------------------------------------------------------------

def _preprocess(cfg, x, edge_index, ppmi_edge_weight, W1, b1, W2, b2,
                dense_w, dense_b):
    n, e = cfg.N, cfg.E
    row = np.asarray(edge_index[0], dtype=np.int64).astype(np.int32)
    col = np.asarray(edge_index[1], dtype=np.int64).astype(np.int32)
    ppmi = np.asarray(ppmi_edge_weight, dtype=np.float64)

    sl = np.arange(n, dtype=np.int32)
    row_sl = np.concatenate([row, sl])
    ones_n = np.ones(n, dtype=np.float64)

    def sym_dis(ew):
        deg = np.bincount(row_sl, weights=ew, minlength=n)
        return np.where(deg > 0, deg ** -0.5, 0.0)

    dis_g = sym_dis(np.concatenate([np.ones(e), ones_n]))
    dis_p = sym_dis(np.concatenate([ppmi, ones_n]))

    # augmented edge list: real edges + self-loop pseudo-edges
    src_a = np.concatenate([row, sl])
    dst_a = np.concatenate([col, sl])
    gn_a = np.concatenate([dis_g[row] * dis_g[col],
                           (dis_g * dis_g)]).astype(np.float32)
    pn_a = np.concatenate([dis_p[row] * ppmi * dis_p[col],
                           (dis_p * dis_p)]).astype(np.float32)

    blk_all = dst_a >> 7
    order = np.lexsort((src_a, blk_all))  # dst block, then ascending src
    src_s = src_a[order].astype(np.int64)
    dst_s = dst_a[order]
    gn_s = gn_a[order]
    pn_s = pn_a[order]
    blk = blk_all[order]
    dstloc = (dst_s & 127).astype(np.float32)

    core_of = blk // cfg.NB
    slot_of = blk - core_of * cfg.NB

    bcnt = np.bincount(blk, minlength=cfg.NBLK_ALL)
    run_start_of_blk = np.concatenate([[0], np.cumsum(bcnt)[:-1]])
    pos = np.arange(src_s.shape[0], dtype=np.int64)

    # Hybrid column assignment:
    #   default: rank-chunks (k-th 128 src-sorted edges) -- zero padding,
    #     cross-core spans are narrow when the 8 blocks sharing a slot have
    #     similar edge counts;
    #   fallback (rare, e.g. the sparse boundary block): 32768-range-split
    #     columns for slots whose cross-core rank-chunk span exceeds the
    #     dma_gather int16 window.
    GUARD = 64
    INF = np.iinfo(np.int64).max
    rank = pos - run_start_of_blk[blk]
    lane = (rank & 127).astype(np.int64)
    kcol = (rank >> 7).astype(np.int64)

    Kblk = -(-bcnt // P)
    Kmat = np.zeros((cfg.ncores, cfg.NB), dtype=np.int64)
    bids = np.arange(cfg.NBLK_ALL)
    Kmat[bids // cfg.NB, bids % cfg.NB] = Kblk[bids]
    K = np.maximum(1, Kmat.max(axis=0))

    K0max = int(K.max())
    cmin = np.full((cfg.NB, K0max), INF, dtype=np.int64)
    cmax = np.full((cfg.NB, K0max), -1, dtype=np.int64)
    np.minimum.at(cmin, (slot_of, kcol), src_s)
    np.maximum.at(cmax, (slot_of, kcol), src_s)
    span = cmax - np.minimum(cmin, cmax)
    badslot = (span > R15 - GUARD).any(axis=1)

    if badslot.any():
        r_e = (src_s >> 15).astype(np.int64)
        cntr = np.bincount(blk * NR + r_e,
                           minlength=cfg.NBLK_ALL * NR).reshape(
                               cfg.NBLK_ALL, NR)
        Cr = np.zeros((cfg.ncores, cfg.NB, NR), dtype=np.int64)
        Cr[bids // cfg.NB, bids % cfg.NB] = -(-cntr[bids] // P)
        Crs = Cr.max(axis=0)  # [NB, NR]
        rb = np.array([0, R15, 2 * R15, 3 * R15, cfg.NPAD], dtype=np.int64)
        for s_ in np.nonzero(badslot)[0]:
            colbase = np.concatenate([[0], np.cumsum(Crs[s_])[:-1]])
            K[s_] = max(1, int(Crs[s_].sum()))
            for c_ in range(cfg.ncores):
                b_ = c_ * cfg.NB + s_
                lo_, n_ = run_start_of_blk[b_], bcnt[b_]
                if n_ == 0:
                    continue
                seg = src_s[lo_:lo_ + n_]
                rr = (seg >> 15).astype(np.int64)
                rstart = np.searchsorted(seg, rb[rr], side="left")
                rank_r = np.arange(n_) - rstart
                kcol[lo_:lo_ + n_] = colbase[rr] + (rank_r >> 7)
                lane[lo_:lo_ + n_] = rank_r & 127

    # per (slot, k) global src min/max across cores (final columns)
    Kmax = int(K.max())
    cmin = np.full((cfg.NB, Kmax), INF, dtype=np.int64)
    cmax = np.full((cfg.NB, Kmax), -1, dtype=np.int64)
    np.minimum.at(cmin, (slot_of, kcol), src_s)
    np.maximum.at(cmax, (slot_of, kcol), src_s)

    # schedule: group columns ordered (k, slot); greedy window passes
    GUARD = 64
    groups = [(g0, min(cfg.GG, cfg.NB - g0))
              for g0 in range(0, cfg.NB, cfg.GG)]
    passes_l = []
    colpos = np.full((cfg.NB, Kmax), -1, dtype=np.int64)
    col = 0
    for (g0, gs) in groups:
        mk = max(int(K[s]) for s in range(g0, g0 + gs))
        gp = []
        cur = None  # [start, ncols, lo, hi]
        for k in range(mk):
            for sl in range(g0, g0 + gs):
                if k >= K[sl]:
                    continue
                colpos[sl][k] = col
                lo, hi = int(cmin[sl][k]), int(cmax[sl][k])
                pad = hi < 0
                if cur is None:
                    cur = [col, 1, lo, hi] if not pad else [col, 1, None,
                                                           None]
                elif pad:
                    cur[1] += 1
                else:
                    nlo = lo if cur[2] is None else min(cur[2], lo)
                    nhi = hi if cur[3] is None else max(cur[3], hi)
                    if nhi - nlo < R15 - GUARD:
                        cur[1] += 1
                        cur[2], cur[3] = nlo, nhi
                    else:
                        gp.append((cur[0], cur[1],
                                   int(cur[2] or 0)))
                        cur = [col, 1, lo, hi]
                col += 1
        assert cur is not None
        gp.append((cur[0], cur[1], int(cur[2] or 0)))
        for (st, nc_, ba) in gp:
            pass  # sanity below
        passes_l.append(tuple(gp))
    NCH = col
    for (g0, gs), gp in zip(groups, passes_l):
        for (st, ncols, base) in gp:
            assert base + R15 <= cfg.NPAD or True
    sched = (tuple(int(v) for v in K), tuple(passes_l))

    # place edges
    abscol = colpos[slot_of, kcol]
    c_arr = core_of.astype(np.int64)

    dst_stream = np.zeros((cfg.ncores, P, NCH), dtype=np.float32)
    nrm_stream = np.zeros((cfg.ncores, P, NCH, 2), dtype=np.float32)
    dst_stream[c_arr, lane, abscol] = dstloc
    nrm_stream[c_arr, lane, abscol, 0] = gn_s
    nrm_stream[c_arr, lane, abscol, 1] = pn_s

    # int16 index stream in dma_gather wrap layout
    pass_start_of_col = np.zeros(NCH, dtype=np.int64)
    pass_base_of_col = np.zeros(NCH, dtype=np.int64)
    for gp in passes_l:
        for (st, ncols, base) in gp:
            pass_start_of_col[st:st + ncols] = st
            pass_base_of_col[st:st + ncols] = base
    idx16 = np.zeros((cfg.ncores, 16, NCH * 8), dtype=np.int16)
    ps_e = pass_start_of_col[abscol]
    rel = src_s - pass_base_of_col[abscol]
    assert (rel >= 0).all() and (rel < R15).all()
    i_flat = (abscol - ps_e) * P + lane
    idx16[c_arr, i_flat % 16, ps_e * 8 + i_flat // 16] = rel.astype(np.int16)
    idx16_full = np.ascontiguousarray(np.tile(idx16, (1, 8, 1)))

    xT = np.zeros((cfg.D, cfg.NPAD), dtype=_FP16)
    xT[:, :n] = np.asarray(x, dtype=np.float32).T.astype(_FP16)

    W1f = np.asarray(W1, dtype=np.float32).astype(_FP16)
    W2f = np.asarray(W2, dtype=np.float32).astype(_FP16)
    b1r2 = np.tile(np.asarray(b1, dtype=np.float32)[None, :], (P, 2))
    dwb = np.tile(np.asarray(dense_w, dtype=np.float32).ravel()[None, :],
                  (P, 1))
    b2r = np.tile(np.asarray(b2, dtype=np.float32)[None, :], (P, 1))

    in_maps = []
    for c in range(cfg.ncores):
        in_maps.append({
            "xT": xT,
            "w1": W1f, "w2": W2f, "b1r2": b1r2, "dwb": dwb, "b2r": b2r,
            "idx16": idx16_full[c], "dsts": dst_stream[c],
            "nrms": nrm_stream[c],
        })
    return in_maps, sched


# ----------------------------------------------------------------------------
# Device program
# ----------------------------------------------------------------------------

def build_program(cfg, sched):
    from concourse import bass, mybir, tile, bacc
    from concourse.masks import make_identity

    dt16 = mybir.dt.float16
    dt32 = mybir.dt.float32
    AOT = mybir.AluOpType
    AFT = mybir.ActivationFunctionType

    groups, passes, block_cols, NCH = _schedule(cfg, sched)
    NB, H, O, D = cfg.NB, cfg.H, cfg.O, cfg.D
    O2 = 2 * O

    nc = bacc.Bacc("TRN2", debug=False, enable_asserts=False,
                   num_devices=cfg.ncores)

    xT = nc.dram_tensor("xT", [D, cfg.NPAD], dt16, kind="ExternalInput")
    w1 = nc.dram_tensor("w1", [D, H], dt16, kind="ExternalInput")
    w2 = nc.dram_tensor("w2", [H, O], dt16, kind="ExternalInput")
    b1r2 = nc.dram_tensor("b1r2", [P, 2 * H], dt32, kind="ExternalInput")
    dwb = nc.dram_tensor("dwb", [P, O], dt32, kind="ExternalInput")
    b2r = nc.dram_tensor("b2r", [P, O], dt32, kind="ExternalInput")
    idx16 = nc.dram_tensor("idx16", [P, NCH * 8], mybir.dt.int16,
                           kind="ExternalInput")
    dsts = nc.dram_tensor("dsts", [P, NCH], dt32, kind="ExternalInput")
    nrms = nc.dram_tensor("nrms", [P, NCH, 2], dt32, kind="ExternalInput")
    outp = nc.dram_tensor("out", [cfg.CORE_ROWS, O], dt32,
                          kind="ExternalOutput")

    hw_full = nc.dram_tensor("hw_full", [cfg.NPAD, H], dt16)
    t_shard = nc.dram_tensor("t_shard", [cfg.CORE_ROWS, O2], dt16)
    t_full = nc.dram_tensor("t_full", [cfg.NPAD, O2], dt16,
                            addr_space="Shared")

    groups_all = [list(range(cfg.ncores))]
    seln = [0]

    with tile.TileContext(nc) as tc:
        with tc.tile_pool(name="const", bufs=1) as cpool:
            w1a = cpool.tile([P, H], dt16)
            w1b = cpool.tile([P, H], dt16)
            nc.sync.dma_start(out=w1a[:], in_=w1[0:P, :])
            nc.sync.dma_start(out=w1b[:], in_=w1[P:2 * P, :])
            w2sb = cpool.tile([P, O], dt16)
            nc.sync.dma_start(out=w2sb[:], in_=w2[:, :])
            b1sb = cpool.tile([P, 2 * H], dt32)
            nc.sync.dma_start(out=b1sb[:], in_=b1r2[:, :])
            dwsb = cpool.tile([P, O], dt32)
            nc.sync.dma_start(out=dwsb[:], in_=dwb[:, :])
            b2sb = cpool.tile([P, O], dt32)
            nc.sync.dma_start(out=b2sb[:], in_=b2r[:, :])
            it16 = cpool.tile([P, P], mybir.dt.int16)
            nc.gpsimd.iota(it16[:], pattern=[[1, P]], base=0,
                           channel_multiplier=0)
            iotaf = cpool.tile([P, P], dt16)
            nc.vector.tensor_copy(out=iotaf[:], in_=it16[:])
            ident16 = cpool.tile([P, P], dt16)
            make_identity(nc, ident16[:])
            idx_sb = cpool.tile([P, NCH * 8], mybir.dt.int16)
            nc.sync.dma_start(out=idx_sb[:], in_=idx16[:, :])
            dst_sb = cpool.tile([P, NCH], dt32)
            nc.sync.dma_start(out=dst_sb[:], in_=dsts[:, :])
            nrm_sb = cpool.tile([P, NCH, 2], dt32)
            nc.sync.dma_start(out=nrm_sb[:], in_=nrms[:, :, :])

            def build_sel(spool, c, br):
                sb = spool.tile([P, P], dt16, tag="sel")
                eng = nc.gpsimd if (seln[0] % 4 == 3) else nc.vector
                seln[0] += 1
                eng.tensor_scalar(
                    out=sb[:], in0=iotaf[:],
                    scalar1=dst_sb[:, c:c + 1],
                    scalar2=nrm_sb[:, c, br:br + 1],
                    op0=AOT.is_equal, op1=AOT.mult)
                return sb

            def gather_group(gpool, gi, table, tbl_rows, tag):
                gp = passes[gi]
                base = gp[0][0]
                kg = sum(ncols for (_, ncols, _) in gp)
                gth = gpool.tile([P, kg, P], dt16, tag=tag)
                for (start, ncols, wbase) in gp:
                    hi = min(wbase + R15, tbl_rows)
                    nc.gpsimd.dma_gather(
                        out_ap=gth[:, start - base:start - base + ncols, :],
                        in_ap=table.ap()[wbase:hi, :],
                        idxs_ap=idx_sb[:, start * 8:(start + ncols) * 8],
                        num_idxs=ncols * P,
                        num_idxs_reg=ncols * P,
                        elem_size=P,
                    )
                return gth, base

            # ------------- phase B: full hw = x @ W1 on every core ----------
            hw_view = hw_full.ap().rearrange("(t p) h -> p t h", p=P)
            with tc.tile_pool(name="phB", bufs=3) as xp, \
                 tc.tile_pool(name="phBp", bufs=4, space="PSUM") as bp, \
                 tc.tile_pool(name="phBh", bufs=2) as hp:
                for g0 in range(0, cfg.NBLK_ALL, cfg.GB):
                    gs = min(cfg.GB, cfg.NBLK_ALL - g0)
                    xa = xp.tile([P, gs * P], dt16, tag="xa")
                    xb = xp.tile([P, gs * P], dt16, tag="xb")
                    nc.sync.dma_start(out=xa[:],
                                      in_=xT[0:P, g0 * P:(g0 + gs) * P])
                    nc.sync.dma_start(out=xb[:],
                                      in_=xT[P:2 * P, g0 * P:(g0 + gs) * P])
                    hwg = hp.tile([P, gs, H], dt16, tag="hwg")
                    for t in range(gs):
                        ps = bp.tile([P, H], dt32, tag="bps")
                        nc.tensor.matmul(out=ps[:],
                                         lhsT=xa[:, t * P:(t + 1) * P],
                                         rhs=w1a[:], start=True, stop=False)
                        nc.tensor.matmul(out=ps[:],
                                         lhsT=xb[:, t * P:(t + 1) * P],
                                         rhs=w1b[:], start=False, stop=True)
                        if t % 2 == 0:
                            nc.scalar.activation(out=hwg[:, t, :], in_=ps[:],
                                                 func=AFT.Copy)
                        else:
                            nc.vector.tensor_copy(out=hwg[:, t, :], in_=ps[:])
                    nc.sync.dma_start(out=hw_view[:, g0:g0 + gs, :], in_=hwg[:])

            # ------------- L1 message pass + on-device @W2 ------------------
            t_view = t_shard.ap().rearrange("(t p) o -> p t o", p=P)
            with tc.tile_pool(name="d_g", bufs=3) as gpool, \
                 tc.tile_pool(name="d_z", bufs=3, space="PSUM") as zpool, \
                 tc.tile_pool(name="d_s", bufs=48) as spool, \
                 tc.tile_pool(name="d_h", bufs=3) as hpool, \
                 tc.tile_pool(name="d_tp", bufs=2, space="PSUM") as tppool, \
                 tc.tile_pool(name="d_tm", bufs=2, space="PSUM") as tmpool, \
                 tc.tile_pool(name="d_o", bufs=2) as opool:
                for gi, (g0, gs) in enumerate(groups):
                    gth, base = gather_group(gpool, gi, hw_full, cfg.NPAD,
                                             "gth1")
                    og = opool.tile([P, gs, O2], dt16, tag="og1")
                    for s in range(gs):
                        blk = g0 + s
                        cols = block_cols[blk]
                        z = zpool.tile([P, 2 * H], dt32, tag="z")
                        for br in range(2):
                            for j, c in enumerate(cols):
                                sb = build_sel(spool, c, br)
                                nc.tensor.matmul(
                                    out=z[:, br * H:(br + 1) * H],
                                    lhsT=sb[:], rhs=gth[:, c - base, :],
                                    start=(j == 0),
                                    stop=(j == len(cols) - 1))
                        hb = hpool.tile([P, 2, H], dt16, tag="hb")
                        nc.vector.tensor_tensor(
                            out=hb[:].rearrange("p b h -> p (b h)"),
                            in0=z[:], in1=b1sb[:], op=AOT.add)
                        hr = hpool.tile([P, 2, H], dt16, tag="hr")
                        nc.scalar.activation(
                            out=hr[:].rearrange("p b h -> p (b h)"),
                            in_=hb[:].rearrange("p b h -> p (b h)"),
                            func=AFT.Relu)
                        tp = tppool.tile([P, 2, P], dt16, tag="tp")
                        nc.tensor.transpose(out=tp[:, 0, :], in_=hr[:, 0, :],
                                            identity=ident16[:])
                        nc.tensor.transpose(out=tp[:, 1, :], in_=hr[:, 1, :],
                                            identity=ident16[:])
                        hT = hpool.tile([P, 2, P], dt16, tag="hT")
                        nc.scalar.activation(
                            out=hT[:].rearrange("p b q -> p (b q)"),
                            in_=tp[:].rearrange("p b q -> p (b q)"),
                            func=AFT.Copy)
                        tm = tmpool.tile([P, 2, O], dt32, tag="tm")
                        nc.tensor.matmul(out=tm[:, 0, :], lhsT=hT[:, 0, :],
                                         rhs=w2sb[:], start=True, stop=True)
                        nc.tensor.matmul(out=tm[:, 1, :], lhsT=hT[:, 1, :],
                                         rhs=w2sb[:], start=True, stop=True)
                        nc.scalar.activation(
                            out=og[:, s, :],
                            in_=tm[:].rearrange("p b o -> p (b o)"),
                            func=AFT.Copy)
                    nc.sync.dma_start(
                        out=t_view[:, g0:g0 + gs, :], in_=og[:])

            nc.gpsimd.collective_compute(
                "AllGather", AOT.bypass, replica_groups=groups_all,
                ins=[t_shard.ap().opt()], outs=[t_full.ap().opt()])

            # ------------- L2 message pass + combine ------------------------
            out_view = outp.ap().rearrange("(t p) o -> p t o", p=P)
            with tc.tile_pool(name="f_g", bufs=3) as gpool, \
                 tc.tile_pool(name="f_z", bufs=3, space="PSUM") as zpool, \
                 tc.tile_pool(name="f_s", bufs=48) as spool, \
                 tc.tile_pool(name="f_e", bufs=4) as epool, \
                 tc.tile_pool(name="f_o", bufs=2) as opool:
                for gi, (g0, gs) in enumerate(groups):
                    gth, base = gather_group(gpool, gi, t_full, cfg.NPAD,
                                             "gth2")
                    og = opool.tile([P, gs, O], dt32, tag="og2")
                    for s in range(gs):
                        blk = g0 + s
                        cols = block_cols[blk]
                        zo = zpool.tile([P, O2], dt32, tag="zo")
                        for br in range(2):
                            for j, c in enumerate(cols):
                                sb = build_sel(spool, c, br)
                                nc.tensor.matmul(
                                    out=zo[:, br * O:(br + 1) * O],
                                    lhsT=sb[:],
                                    rhs=gth[:, c - base,
                                            br * O:(br + 1) * O],
                                    start=(j == 0),
                                    stop=(j == len(cols) - 1))
                        zps = epool.tile([P, O], dt32, tag="zps")
                        nc.scalar.activation(out=zps[:], in_=zo[:, O:O2],
                                             func=AFT.Copy)
                        zd = epool.tile([P, O], dt32, tag="zd")
                        nc.vector.tensor_tensor(out=zd[:], in0=zo[:, 0:O],
                                                in1=zps[:],
                                                op=AOT.subtract)
                        scr = epool.tile([P, O], dt32, tag="scr")
                        dl = epool.tile([P, 1], dt32, tag="dl")
                        nc.vector.tensor_tensor(out=scr[:], in0=zd[:],
                                                in1=dwsb[:], op=AOT.mult)
                        nc.vector.tensor_reduce(out=dl[:], in_=scr[:],
                                                axis=mybir.AxisListType.X,
                                                op=AOT.add)
                        wg = epool.tile([P, 1], dt32, tag="wg")
                        nc.scalar.activation(out=wg[:], in_=dl[:],
                                             func=AFT.Sigmoid)
                        bl = epool.tile([P, O], dt32, tag="bl")
                        nc.vector.scalar_tensor_tensor(
                            out=bl[:], in0=zd[:], scalar=wg[:],
                            in1=zps[:], op0=AOT.mult, op1=AOT.add)
                        nc.vector.tensor_tensor(out=og[:, s, :], in0=bl[:],
                                                in1=b2sb[:], op=AOT.add)
                    nc.sync.dma_start(out=out_view[:, g0:g0 + gs, :],
                                      in_=og[:])

    nc.compile()
    return nc


_BUILD_CACHE = {}


def _get_program(cfg, sched):
    key = (cfg.N, cfg.E, cfg.GG, sched)
    if key not in _BUILD_CACHE:
        _BUILD_CACHE[key] = build_program(cfg, sched)
    return _BUILD_CACHE[key]


LAST_RESULTS = None


def _run(cfg, inputs):
    from concourse.bass_utils import run_bass_kernel_spmd
    global LAST_RESULTS
    in_maps, sched = _preprocess(cfg, **inputs)
    nc = _get_program(cfg, sched)
    trace = bool(int(os.environ.get("KERNEL_TRACE", "0")))
    res = run_bass_kernel_spmd(nc, in_maps, core_ids=list(range(cfg.ncores)),
                               trace=trace)
    LAST_RESULTS = res
    out = np.concatenate([res.results[c]["out"] for c in range(cfg.ncores)],
                         axis=0)[:cfg.N]
    return np.ascontiguousarray(out.astype(np.float32))


def kernel(x, edge_index, ppmi_edge_weight, W1, b1, W2, b2, dense_w, dense_b):
    return _run(FULL, dict(x=x, edge_index=edge_index,
                           ppmi_edge_weight=ppmi_edge_weight, W1=W1, b1=b1,
                           W2=W2, b2=b2, dense_w=dense_w, dense_b=dense_b))
